# revision 41
# baseline (speedup 1.0000x reference)
"""Single-head attention (B=8, S=2048, D_in=D_out=1024) on 8 Trainium2 NeuronCores.

Sharding: data-parallel over batch - core b computes batch element b end-to-end.

Design (vs the fp32r baseline, measured 450-581 us): all matmuls run in fp16
(1 cyc/row on the PE, same rate as fp32r, but 16-bit operands halve SBUF and
drain bytes and make X^T transposes 1 cyc/row instead of fp32's 2), and the
algebra is restructured to cut PE rows and kill phase-B transposes:

  A    = Wq @ Wk^T                 (weight fusion: replaces the K projection;
                                    computed per pass, W^T 33K + A 66K rows
                                    vs 131K for the K projection)
  T^T  = A-stationary @ Xq^T       (= (Xq A)^T, replaces the Q projection)
  S^T  = Xk^T-stationary @ T^T     (scores, transposed: the exp output P^T is
                                    then directly the PV stationary -> NO PE
                                    transposes of P in phase B)
  P^T  = exp(S^T/32 - 10)          (ACT, fp16 out; the -10 shift keeps the
                                    unnormalized exp in fp16 range and cancels
                                    in the final normalization)
  V    = Xv^T-stationary @ Wv
  Z    = P^T-stationary @ V        (+ per-query-tile rowsum chains against a
                                    ones column, N=1 matmuls)
  z    = Z * (1/rowsum)            (DVE reciprocal + scale in the PSUM drain)

Schedule notes (sim-guided, each validated on HW):
- Phase A is one PE-dense pipeline: per seq tile, xv/xk/xq and one W d-tile
  land (DMA fp32), cast to fp16 (ACT/DVE), and their PE-transposes (fp16,
  1 cyc/row, batched per PSUM bank with one strided drain) interleave with
  the V-projection matmuls so the PE never waits on the DMA+cast chains.
  A separate W^T stretch would be DMA-bound and expose ~12 us. A closes
  phase A.
- Staging pools (xf/x16/wf) live outside the rep scope so the next rep's
  input DMAs prefetch during this rep's phase B.
- Input DMAs ride the SP queue; z outputs ride the scalar queue. Sharing one
  queue head-of-line-blocks the next rep's prefetch behind z writes that
  complete only at rep end (cost ~6 us/rep on HW).

PE row budget/pass: W^T 16K(fp16) + A 66K + X^T 49K + V 131K + T^T 131K +
S^T 262K + PV 262K + rowsums ~15K ~= 933K cyc @ 2.4 GHz ~= 389 us; HW
measures 313-400 us/pass depending on device co-tenancy (slope method, best
313 us), cost-model sim 417 us/rep, PE busy ~93%. Remaining known headroom:
offloading the X^T/W^T transposes to DMA-xbar via an fp16 DRAM scratch
(~27 us of PE) -- see kernel_v5_wip.py; blocked on DMA-queue orchestration.

Numerics: everything through the PE is round-to-nearest fp16 with fp32 PSUM
accumulation; end-to-end rel err vs the fp32 reference is 1.07e-3 (the
fp32r baseline measured 5.4e-4; the gate is 2e-2).
"""

from contextlib import ExitStack

import numpy as np

import concourse.bacc as bacc
import concourse.mybir as mybir
import concourse.tile as tile
from concourse.masks import make_identity

F32 = mybir.dt.float32
F16 = mybir.dt.float16

B, S, D = 8, 2048, 1024
P = 128                    # SBUF partitions
TD = D // P                # 8 d/e tiles
TS = S // P                # 16 seq tiles
IC = 512                   # phase-B query chunk
NIC = S // IC              # 4
EC = 512                   # value-dim chunk
NEC = D // EC              # 2
SCALE = 1.0 / float(np.sqrt(D))
ESHIFT = -10.0             # exp shift; cancels in normalization


def build_program(repeats: int = 1, phases: str = "ab"):
    nc = bacc.Bacc("TRN2", target_bir_lowering=False, debug=False)

    xk = nc.dram_tensor("xk", [S, D], F32, kind="ExternalInput").ap()
    xv = nc.dram_tensor("xv", [S, D], F32, kind="ExternalInput").ap()
    xq = nc.dram_tensor("xq", [S, D], F32, kind="ExternalInput").ap()
    wk = nc.dram_tensor("wk", [D, D], F32, kind="ExternalInput").ap()
    wv = nc.dram_tensor("wv", [D, D], F32, kind="ExternalInput").ap()
    wq = nc.dram_tensor("wq", [D, D], F32, kind="ExternalInput").ap()
    z = nc.dram_tensor("z", [S, D], F32, kind="ExternalOutput").ap()

    with tile.TileContext(nc) as tc, ExitStack() as ctx:
        top = ctx.enter_context(tc.tile_pool(name="top", bufs=1))
        ident32 = top.tile([P, P], F32, tag="id32", name="id32")
        make_identity(nc, ident32[:])
        ident16 = top.tile([P, P], F16, tag="id16", name="id16")
        make_identity(nc, ident16[:])
        ones16 = top.tile([P, 1], F16, tag="ones", name="ones")
        nc.vector.memset(ones16[:], 1.0)
        eshift = top.tile([P, 1], F32, tag="eshift", name="eshift")
        nc.vector.memset(eshift[:], ESHIFT)

        # staging pools live OUTSIDE the rep scope so the next rep's DMAs can
        # prefetch into them while the current rep's phase B runs.
        xf = ctx.enter_context(tc.tile_pool(name="xf", bufs=4))
        x16p = ctx.enter_context(tc.tile_pool(name="x16", bufs=8))
        wf = ctx.enter_context(tc.tile_pool(name="wf", bufs=2))

        for rep in range(repeats):
            _one_pass(nc, tc, ident32, ident16, ones16, eshift,
                      xf, x16p, wf, xk, xv, xq, wk, wv, wq, z, rep)

    nc.compile()
    return nc


def _one_pass(nc, tc, id32, id16, ones, eshift, xf, x16p, wf,
              xk, xv, xq, wk, wv, wq, z, rep):
    # All input DMAs go on the sync (SP) queue; z outputs alone use the
    # scalar queue. Sharing a queue would head-of-line-block the next
    # rep's input prefetch behind this rep's z writes (which complete only
    # at rep end), and the SP sequencer has nothing else to do.
    qs = [nc.sync, nc.sync]

    with tc.tile_pool(name=f"res{rep}", bufs=1) as res:
        # residents (per-partition): 32+32+32+16+16 = 128 KB
        xkT = res.tile([P, TD, S], F16, tag="xkT", name="xkT")
        xqT = res.tile([P, TD, S], F16, tag="xqT", name="xqT")
        vres = res.tile([P, TS, D], F16, tag="vres", name="vres")
        a_res = res.tile([P, TD, D], F16, tag="a", name="a_res")
        wv16 = res.tile([P, TD, D], F16, tag="wv16", name="wv16")

        # ---------- phase A ----------
        # PE order: [xv transposes + V proj] (dense, hides xk/xq/w DMA+casts)
        # -> [W^T transposes + A build] -> [xk/xq transposes].
        with (
            tc.tile_pool(name=f"wst{rep}", bufs=1) as wst,
            tc.tile_pool(name=f"psxt{rep}", bufs=3, space="PSUM") as psxt,
            tc.tile_pool(name=f"psv{rep}", bufs=3, space="PSUM") as psv,
        ):
            # wv: plain cast fp32 -> fp16 (needed first, by the V projection)
            for dt in range(TD):
                f = wf.tile([P, D], F32, tag="wf32", name="wf32")
                qs[dt % 2].dma_start(f[:], wv[dt * P : (dt + 1) * P, :])
                nc.any.tensor_copy(wv16[:, dt], f[:])

            def land_x(x_dram, st):
                f = xf.tile([P, D], F32, tag="xf32", name="xf32")
                qs[st % 2].dma_start(f[:], x_dram[st * P : (st + 1) * P, :])
                x16 = x16p.tile([P, D], F16, tag="x16", name="x16")
                nc.any.tensor_copy(x16[:], f[:])
                return x16

            def xT_tile(x16, dest, st):
                bank = psxt.tile([P, TD * P], F16, tag="xtp", name="xtp")
                for d in range(TD):
                    nc.tensor.transpose(
                        bank[:, d * P : (d + 1) * P],
                        x16[:, d * P : (d + 1) * P],
                        id16[:],
                    )
                nc.any.tensor_copy(
                    dest[:, :, st * P : (st + 1) * P],
                    bank[:].rearrange("p (d c) -> p d c", c=P),
                )

            wqT = wst.tile([P, TD, D], F16, tag="wqT", name="wqT")
            wkT = wst.tile([P, TD, D], F16, tag="wkT", name="wkT")

            # X landing + all X^T transposes + V projection + W^T, one
            # PE-dense pipeline: per seq tile the PE does xv/xk/xq/w
            # transposes (~1.7us) + V-proj matmuls (3.4us), hiding the 4
            # DMA+cast chains. (One W d-tile rides along with each seq tile;
            # a separate W^T stretch would be DMA-bound and expose ~12us.)
            with tc.tile_pool(name=f"xvp{rep}", bufs=3) as xvp:
                for st in range(TS):
                    x16v = land_x(xv, st)
                    x16k = land_x(xk, st)
                    x16q = land_x(xq, st)
                    wdram, wT = (wq, wqT) if st < TD else (wk, wkT)
                    dt = st % TD
                    fw = wf.tile([P, D], F32, tag="wf32", name="wf32")
                    qs[st % 2].dma_start(fw[:], wdram[dt * P : (dt + 1) * P, :])
                    w16 = x16p.tile([P, D], F16, tag="x16", name="x16")
                    nc.any.tensor_copy(w16[:], fw[:])
                    xvT = xvp.tile([P, TD, P], F16, tag="xvT", name="xvT")
                    bank = psxt.tile([P, TD * P], F16, tag="xtp", name="xtp")
                    for d in range(TD):
                        nc.tensor.transpose(
                            bank[:, d * P : (d + 1) * P],
                            x16v[:, d * P : (d + 1) * P],
                            id16[:],
                        )
                    nc.any.tensor_copy(
                        xvT[:], bank[:].rearrange("p (d c) -> p d c", c=P)
                    )
                    for ec in range(NEC):
                        ps = psv.tile([P, EC], F32, tag="vps", name="vps")
                        for d in range(TD):
                            nc.tensor.matmul(
                                ps[:],
                                xvT[:, d],
                                wv16[:, d, ec * EC : (ec + 1) * EC],
                                start=(d == 0),
                                stop=(d == TD - 1),
                            )
                        nc.any.tensor_copy(
                            vres[:, st, ec * EC : (ec + 1) * EC], ps[:]
                        )
                    xT_tile(x16k, xkT, st)
                    xT_tile(x16q, xqT, st)
                    for eb in range(TD // 4):
                        bank = psxt.tile(
                            [P, 4 * P], F16, tag="wtp", name="wtp", bufs=2
                        )
                        for k in range(4):
                            et = eb * 4 + k
                            nc.tensor.transpose(
                                bank[:, k * P : (k + 1) * P],
                                w16[:, et * P : (et + 1) * P],
                                id16[:],
                            )
                        nc.any.tensor_copy(
                            wT[:, eb * 4 : (eb + 1) * 4, dt * P : (dt + 1) * P],
                            bank[:].rearrange("p (e c) -> p e c", c=P),
                        )

            # A = Wq @ Wk^T (W^T produced inside the pipeline above)
            for dt in range(TD):
                for ch in range(NEC):
                    ps = psv.tile([P, EC], F32, tag="vps", name="vps")
                    for e in range(TD):
                        nc.tensor.matmul(
                            ps[:],
                            wqT[:, e, dt * P : (dt + 1) * P],
                            wkT[:, e, ch * EC : (ch + 1) * EC],
                            start=(e == 0),
                            stop=(e == TD - 1),
                        )
                    nc.any.tensor_copy(a_res[:, dt, ch * EC : (ch + 1) * EC], ps[:])



        # ---------- phase B: attention ----------
        with (
            tc.tile_pool(name=f"tt{rep}", bufs=2) as ttp,
            tc.tile_pool(name=f"pt{rep}", bufs=1) as ptp,
            tc.tile_pool(name=f"zo{rep}", bufs=2) as zop,
            tc.tile_pool(name=f"rc{rep}", bufs=2) as rcp,
            tc.tile_pool(name=f"psb{rep}", bufs=2, space="PSUM") as psb,
        ):
            for ic in range(NIC):
                # T^T(ic) = A-stationary @ Xq^T(ic)
                tT = ttp.tile([P, TD, IC], F16, tag="tT", name="tT")
                for dc in range(TD):
                    ps = psb.tile([P, IC], F32, tag="st", name="st_ps", bufs=3)
                    for d in range(TD):
                        nc.tensor.matmul(
                            ps[:],
                            a_res[:, d, dc * P : (dc + 1) * P],
                            xqT[:, d, ic * IC : (ic + 1) * IC],
                            start=(d == 0),
                            stop=(d == TD - 1),
                        )
                    nc.any.tensor_copy(tT[:, dc], ps[:])
                # S^T(j, ic) = Xk^T(j)-stationary @ T^T ; P^T = exp
                pT = ptp.tile([P, TS, IC], F16, tag="pT", name="pT")
                for j in range(TS):
                    ps = psb.tile([P, IC], F32, tag="st", name="st_ps", bufs=3)
                    for dc in range(TD):
                        nc.tensor.matmul(
                            ps[:],
                            xkT[:, dc, j * P : (j + 1) * P],
                            tT[:, dc],
                            start=(dc == 0),
                            stop=(dc == TD - 1),
                        )
                    nc.scalar.activation(
                        pT[:, j],
                        ps[:],
                        mybir.ActivationFunctionType.Exp,
                        scale=SCALE,
                        bias=eshift[:],
                    )
                # PV + rowsum chains per 128-query tile
                for it in range(IC // P):
                    zp = psb.tile([P, D], F32, tag="zp", name="zp", bufs=2)
                    rs = psb.tile([P, 1], F32, tag="rs", name="rs", bufs=1)
                    for j in range(TS):
                        lhs = pT[:, j, it * P : (it + 1) * P]
                        nc.tensor.matmul(
                            zp[:, 0:EC], lhs, vres[:, j, 0:EC],
                            start=(j == 0), stop=(j == TS - 1),
                        )
                        nc.tensor.matmul(
                            zp[:, EC:D], lhs, vres[:, j, EC:D],
                            start=(j == 0), stop=(j == TS - 1),
                        )
                        nc.tensor.matmul(
                            rs[:], lhs, ones[:, 0:1],
                            start=(j == 0), stop=(j == TS - 1),
                        )
                    rec = rcp.tile([P, 1], F32, tag="rec", name="rec")
                    nc.vector.reciprocal(rec[:], rs[:])
                    row = (ic * (IC // P) + it) * P
                    for ec in range(NEC):
                        zo = zop.tile([P, EC], F32, tag="zo", name="zo")
                        nc.vector.tensor_scalar_mul(
                            zo[:], zp[:, ec * EC : (ec + 1) * EC], rec[:]
                        )
                        nc.scalar.dma_start(
                            z[row : row + P, ec * EC : (ec + 1) * EC], zo[:]
                        )


_EXEC = None
_EXEC_BODY = None


def _build_exec(nc=None):
    """Compile the per-core program and wrap it in one jitted 8-core SPMD
    callable (shard_map over the 8 NeuronCores)."""
    import jax
    from jax.experimental.shard_map import shard_map
    from jax.sharding import Mesh, PartitionSpec

    from concourse import bass2jax

    if nc is None:
        nc = build_program()
    bass2jax.install_neuronx_cc_hook()

    partition_name = nc.partition_id_tensor.name if nc.partition_id_tensor else None
    in_names, out_names, out_avals, zero_outs = [], [], [], []
    for alloc in nc.m.functions[0].allocations:
        if not isinstance(alloc, mybir.MemoryLocationSet):
            continue
        name = alloc.memorylocations[0].name
        if alloc.kind == "ExternalInput":
            if name != partition_name:
                in_names.append(name)
        elif alloc.kind == "ExternalOutput":
            assert alloc.tensor_shape is not None and alloc.dtype is not None
            out_names.append(name)
            shape = tuple(alloc.tensor_shape)
            dtype = mybir.dt.np(alloc.dtype)
            out_avals.append(jax.core.ShapedArray(shape, dtype))
            zero_outs.append(np.zeros(shape, dtype))
    n_params = len(in_names)
    all_in_names = tuple(in_names) + tuple(out_names)
    if partition_name is not None:
        all_in_names = all_in_names + (partition_name,)

    def _body(*args):
        operands = list(args)
        if partition_name is not None:
            operands.append(bass2jax.partition_id_tensor())
        outs = bass2jax._bass_exec_p.bind(
            *operands,
            out_avals=tuple(out_avals),
            in_names=all_in_names,
            out_names=tuple(out_names),
            lowering_input_output_aliases=(),
            sim_require_finite=True,
            sim_require_nnan=True,
            nc=nc,
        )
        return tuple(outs)

    devices = jax.devices()[:B]
    assert len(devices) == B, f"need {B} cores, have {len(jax.devices())}"
    mesh = Mesh(np.asarray(devices), ("core",))
    n_outs = len(out_names)
    sharded_body = shard_map(
        _body,
        mesh=mesh,
        in_specs=(PartitionSpec("core"),) * (n_params + n_outs),
        out_specs=(PartitionSpec("core"),) * n_outs,
        check_rep=False,
    )
    global _EXEC_BODY
    _EXEC_BODY = sharded_body
    fn = jax.jit(sharded_body, keep_unused=True)
    return fn, mesh, in_names, out_names, zero_outs


def _get_exec():
    global _EXEC
    if _EXEC is None:
        _EXEC = _build_exec()
    return _EXEC


def _concat_inputs(in_maps):
    """Per-core input dicts -> global concat arrays in executable order."""
    fn, mesh, in_names, out_names, zero_outs = _get_exec()
    concat_in = [
        np.concatenate([in_maps[c][name] for c in range(B)], axis=0)
        for name in in_names
    ]
    concat_zeros = [
        np.zeros((B * z.shape[0], *z.shape[1:]), z.dtype) for z in zero_outs
    ]
    return concat_in + concat_zeros


def kernel(
    inputs_for_keys: np.ndarray,
    inputs_for_values: np.ndarray,
    inputs_for_queries: np.ndarray,
    W_K: np.ndarray,
    W_V: np.ndarray,
    W_Q: np.ndarray,
) -> np.ndarray:
    fn, mesh, in_names, out_names, zero_outs = _get_exec()
    wk_ = np.ascontiguousarray(W_K, dtype=np.float32)
    wv_ = np.ascontiguousarray(W_V, dtype=np.float32)
    wq_ = np.ascontiguousarray(W_Q, dtype=np.float32)
    in_maps = [
        {
            "xk": np.ascontiguousarray(inputs_for_keys[b], dtype=np.float32),
            "xv": np.ascontiguousarray(inputs_for_values[b], dtype=np.float32),
            "xq": np.ascontiguousarray(inputs_for_queries[b], dtype=np.float32),
            "wk": wk_,
            "wv": wv_,
            "wq": wq_,
        }
        for b in range(B)
    ]
    out_arrs = fn(*_concat_inputs(in_maps))
    z_all = np.asarray(out_arrs[out_names.index("z")])
    return z_all.reshape(B, S, D)


if __name__ == "__main__":
    rng = np.random.default_rng(0)
    ins = {
        "inputs_for_keys": rng.standard_normal((B, S, D), dtype=np.float32),
        "inputs_for_values": rng.standard_normal((B, S, D), dtype=np.float32),
        "inputs_for_queries": rng.standard_normal((B, S, D), dtype=np.float32),
        "W_K": (rng.standard_normal((D, D)) * 0.05).astype(np.float32),
        "W_V": (rng.standard_normal((D, D)) * 0.05).astype(np.float32),
        "W_Q": (rng.standard_normal((D, D)) * 0.05).astype(np.float32),
    }
    out = kernel(**ins)
    print("out", out.shape, out.dtype)
